# revision 1
# baseline (speedup 1.0000x reference)
"""AdditiveUniAttention kernel for 8 TRN2 NeuronCores.

Strategy: pure data-parallel over B (8 batch elements, 8 cores, no collectives).

Per-core math (b fixed):
  QeT2[j*64+a, m] = sum_h meta[m,h] Wq_w[a,h]          (dup over j=0,1)
  Qbias[j*64+a, p] = QeT2[j*64+a, 2p+j] + Wq_b[a] + Wk_b[a]
  Ke2[j*64+a, l]  = sum_h text[l,h] Wk_w[a,h]          (dup over j=0,1)
  T_p[(j,a), l]   = tanh(Ke2[(j,a), l] + Qbias[(j,a), p])       (ScalarE bias)
  scores[m, l]    = sum_a v[a] T_{m//2}[(m%2,a), l]    (block-column lhsT)
                    + (mask[l]-1)*1e30                  (rank-1 matmul)
  exp = exp(scores) (f32 PSUM -> bf16 SBUF), sumexp via accum_out
  D[m, h]   = sum_l exp^T[l, m] text[l, h]   (PE transposes of exp rows)
  context   = (D / sumexp) @ Vv_w^T + Vv_b   (transposed weights from host)
  out       = LayerNorm(meta + context) * g + b

Matmuls all run bf16 (1 cyc/col on PE vs 4 for f32) with f32 PSUM accumulation.
Softmax/LN math in f32.  Weight-layout prep (transposes, bf16 casts, the
block-column v matrix) happens host-side in numpy; all activation-tensor
arithmetic happens on device.
"""

import numpy as np
import ml_dtypes
from contextlib import ExitStack

import concourse.bass as bass
import concourse.tile as tile
import concourse.mybir as mybir
from concourse import bacc
from concourse.bass_utils import run_bass_kernel_spmd

BF16 = ml_dtypes.bfloat16
F32 = mybir.dt.float32
BF = mybir.dt.bfloat16
I32 = mybir.dt.int32

B, M, L, H, A = 8, 64, 2048, 768, 64
NCORES = 8
LH = L // 2          # l-half size (1024)
NP = M // 2          # m-pairs (32)
HC = H // 128        # h chunks (6)
AF = mybir.ActivationFunctionType


def _emit(ctx, tc, nc, d):
    """Emit the per-core kernel body under TileContext tc."""
    const = ctx.enter_context(tc.tile_pool(name="const", bufs=1))
    work = ctx.enter_context(tc.tile_pool(name="work", bufs=1))
    tpool = ctx.enter_context(tc.tile_pool(name="tpool", bufs=3))
    pbig = ctx.enter_context(tc.tile_pool(name="pbig", bufs=2, space="PSUM"))
    psml = ctx.enter_context(tc.tile_pool(name="psml", bufs=2, space="PSUM"))
    pout = ctx.enter_context(tc.tile_pool(name="pout", bufs=1, space="PSUM"))

    # --- dummy tanh: pull the exp_and_others ACT table load to t=0
    dmy = const.tile([128, 1], F32)
    nc.vector.memset(dmy[:], 0.0)
    nc.scalar.activation(dmy[:], dmy[:], AF.Tanh)

    # --- critical-path DMAs first: textT half 0
    textT_sb = []
    for hc in range(HC):
        t_tT = const.tile([128, L], BF, name=f"textT{hc}", tag=f"textT{hc}")
        textT_sb.append(t_tT)
    for hc in range(HC):
        nc.sync.dma_start(
            textT_sb[hc][:, 0:LH], d["textT"][hc * 128:(hc + 1) * 128, 0:LH]
        )

    # --- small weights / meta
    wk2_sb = const.tile([128, HC * 128], BF)
    nc.sync.dma_start(
        wk2_sb[:].rearrange("p (c m) -> p c m", c=HC),
        d["wk2"].rearrange("(c p) m -> p c m", p=128),
    )
    wqt2_sb = const.tile([128, HC * 128], BF)
    nc.sync.dma_start(
        wqt2_sb[:].rearrange("p (c m) -> p c m", c=HC),
        d["wqt2"].rearrange("(c p) m -> p c m", p=128),
    )
    metaT_sb = const.tile([128, HC * 64], BF)
    nc.sync.dma_start(
        metaT_sb[:].rearrange("p (c m) -> p c m", c=HC),
        d["metaT"].rearrange("(c p) m -> p c m", p=128),
    )
    lt_sb = const.tile([128, NP * 64], BF)
    nc.sync.dma_start(lt_sb[:], d["lt"][:, :])
    bias2_sb = const.tile([128, 1], F32)
    nc.sync.dma_start(bias2_sb[:], d["bias2"][:, :])
    ones64_sb = const.tile([1, 64], BF)
    nc.sync.dma_start(ones64_sb[:], d["ones64"][:, :])
    id64_sb = const.tile([64, 64], BF)
    nc.sync.dma_start(id64_sb[:], d["id64"][:, :])
    mask_sb = work.tile([1, L], I32)
    nc.sync.dma_start(mask_sb[:], d["mask"][:, :])

    # --- textT half 1, then the natural-layout text, then tail weights
    for hc in range(HC):
        nc.sync.dma_start(
            textT_sb[hc][:, LH:L], d["textT"][hc * 128:(hc + 1) * 128, LH:L]
        )
    tb_sb = const.tile([128, 16 * H], BF)   # text natural: [l%128, (l//128, h)]
    for g in range(4):
        nc.sync.dma_start(
            tb_sb[:, g * 4 * H:(g + 1) * 4 * H].rearrange("p (c m) -> p c m", c=4),
            d["textn"][g * 512:(g + 1) * 512, :].rearrange("(c p) m -> p c m", p=128),
        )
    vvt_sb = const.tile([128, HC * H], BF)
    nc.sync.dma_start(
        vvt_sb[:].rearrange("p (c m) -> p c m", c=HC),
        d["vvt"].rearrange("(c p) m -> p c m", p=128),
    )
    vvb_sb = const.tile([1, H], BF)
    nc.sync.dma_start(vvb_sb[:], d["vvb"][:, :])
    meta_sb = const.tile([64, H], F32)
    nc.sync.dma_start(meta_sb[:], d["meta"][:, :])
    gbc_sb = const.tile([64, H], F32)
    nc.sync.dma_start(gbc_sb[:], d["gbc"][:, :])
    bbc_sb = const.tile([64, H], F32)
    nc.sync.dma_start(bbc_sb[:], d["bbc"][:, :])

    # --- Qe path: QeT2 = wqt2^T-chunks @ metaT-chunks, then Qbias assembly
    qe_ps = psml.tile([128, 64], F32, tag="sm")
    for hc in range(HC):
        nc.tensor.matmul(
            qe_ps[:],
            lhsT=wqt2_sb[:, hc * 128:(hc + 1) * 128],
            rhs=metaT_sb[:, hc * 64:(hc + 1) * 64],
            start=(hc == 0),
            stop=(hc == HC - 1),
        )
    qet2_sb = work.tile([128, 64], F32)
    nc.vector.tensor_copy(qet2_sb[:], qe_ps[:])
    qbias = work.tile([128, NP], F32)
    nc.vector.tensor_scalar(
        qbias[0:64, :], qet2_sb[0:64, 0:64:2], bias2_sb[0:64, :], None,
        mybir.AluOpType.add,
    )
    nc.vector.tensor_scalar(
        qbias[64:128, :], qet2_sb[64:128, 1:64:2], bias2_sb[64:128, :], None,
        mybir.AluOpType.add,
    )

    # --- main loop over l-halves
    exp_bf = work.tile([64, L], BF)
    ke2_sb = work.tile([128, L], F32)
    attnT_sb = work.tile([128, 16 * 64], BF)
    d_ps = pout.tile([64, H], F32, tag="o")
    sumexp = []
    for half in range(2):
        l0 = half * LH
        # Ke2 for this half: accumulate over h-chunks into PSUM, copy to SBUF
        ke_ps = pbig.tile([128, LH], F32, tag="big", name=f"ke_ps{half}")
        for hc in range(HC):
            for j in range(2):
                nc.tensor.matmul(
                    ke_ps[:, j * 512:(j + 1) * 512],
                    lhsT=wk2_sb[:, hc * 128:(hc + 1) * 128],
                    rhs=textT_sb[hc][:, l0 + j * 512:l0 + (j + 1) * 512],
                    start=(hc == 0),
                    stop=(hc == HC - 1),
                )
        nc.vector.tensor_copy(ke2_sb[:, l0:l0 + LH], ke_ps[:])

        # tanh + score matmuls
        sc_ps = pbig.tile([64, LH], F32, tag="big", name=f"sc_ps{half}")
        for p in range(NP):
            tt = tpool.tile([128, LH], BF, name="tt", tag="tt")
            nc.scalar.activation(
                tt[:], ke2_sb[:, l0:l0 + LH], AF.Tanh, bias=qbias[:, p:p + 1]
            )
            for j in range(2):
                nc.tensor.matmul(
                    sc_ps[:, j * 512:(j + 1) * 512],
                    lhsT=lt_sb[:, p * 64:(p + 1) * 64],
                    rhs=tt[:, j * 512:(j + 1) * 512],
                    start=(p == 0),
                    stop=False,
                )
        if half == 0:
            # mask row prep (off ACT critical path; emitted late so the DVE
            # runs the Ke2/Qbias copies first)
            mrow_f = work.tile([1, L], F32)
            nc.vector.tensor_copy(mrow_f[:], mask_sb[:])
            mrow_s = work.tile([1, L], F32)
            nc.vector.tensor_scalar(
                mrow_s[:], mrow_f[:], -1.0, 1.0e30,
                mybir.AluOpType.add, mybir.AluOpType.mult,
            )
            mrow_bf = work.tile([1, L], BF)
            nc.vector.tensor_copy(mrow_bf[:], mrow_s[:])
        for j in range(2):
            nc.tensor.matmul(
                sc_ps[:, j * 512:(j + 1) * 512],
                lhsT=ones64_sb[:],
                rhs=mrow_bf[0:1, l0 + j * 512:l0 + (j + 1) * 512],
                start=False,
                stop=True,
            )

        # exp (+ row sums), straight out of PSUM
        se = work.tile([64, 1], F32, name=f"se{half}", tag=f"se{half}")
        nc.scalar.activation(
            exp_bf[:, l0:l0 + LH], sc_ps[:], AF.Exp, accum_out=se[:]
        )
        sumexp.append(se)

        # attn^T chunks (PE transpose) + context matmul D += attnT^T @ text
        for k in range(half * 8, half * 8 + 8):
            tp = psml.tile([128, 64], BF, tag="sm", name="tp")
            nc.tensor.transpose(tp[:], exp_bf[:, k * 128:(k + 1) * 128], id64_sb[:])
            nc.vector.tensor_copy(attnT_sb[:, k * 64:(k + 1) * 64], tp[:])
            for n0, nw in ((0, 512), (512, 256)):
                nc.tensor.matmul(
                    d_ps[:, n0:n0 + nw],
                    lhsT=attnT_sb[:, k * 64:(k + 1) * 64],
                    rhs=tb_sb[:, k * H + n0:k * H + n0 + nw],
                    start=(k == 0),
                    stop=(k == 15),
                )

    # --- epilogue
    s_all = work.tile([64, 1], F32)
    nc.vector.tensor_add(s_all[:], sumexp[0][:], sumexp[1][:])
    sinv = work.tile([64, 1], F32)
    nc.vector.reciprocal(sinv[:], s_all[:])
    d_sb = work.tile([64, H], BF)
    nc.vector.tensor_scalar(
        d_sb[:], d_ps[:], sinv[:], None, mybir.AluOpType.mult
    )
    dt_sb = work.tile([128, HC * 64], BF)
    for hc in range(HC):
        dtp = psml.tile([128, 64], BF, tag="sm", name="dtp")
        nc.tensor.transpose(dtp[:], d_sb[:, hc * 128:(hc + 1) * 128], id64_sb[:])
        nc.vector.tensor_copy(dt_sb[:, hc * 64:(hc + 1) * 64], dtp[:])
    ctx_ps = pout.tile([64, H], F32, tag="o")
    for hc in range(HC):
        for n0, nw in ((0, 512), (512, 256)):
            nc.tensor.matmul(
                ctx_ps[:, n0:n0 + nw],
                lhsT=dt_sb[:, hc * 64:(hc + 1) * 64],
                rhs=vvt_sb[:, hc * H + n0:hc * H + n0 + nw],
                start=(hc == 0),
                stop=False,
            )
    for n0, nw in ((0, 512), (512, 256)):
        nc.tensor.matmul(
            ctx_ps[:, n0:n0 + nw],
            lhsT=ones64_sb[:],
            rhs=vvb_sb[0:1, n0:n0 + nw],
            start=False,
            stop=True,
        )

    # x = meta + context;  LayerNorm
    x_sb = work.tile([64, H], F32)
    nc.vector.tensor_add(x_sb[:], meta_sb[:], ctx_ps[:])
    xsum = work.tile([64, 1], F32)
    nc.vector.reduce_sum(xsum[:], x_sb[:], axis=mybir.AxisListType.X)
    mu = work.tile([64, 1], F32)
    nc.vector.tensor_scalar(mu[:], xsum[:], 1.0 / H, None, mybir.AluOpType.mult)
    xc = work.tile([64, H], F32)
    nc.vector.tensor_scalar(xc[:], x_sb[:], mu[:], None, mybir.AluOpType.subtract)
    sq = work.tile([64, H], F32)
    varsum = work.tile([64, 1], F32)
    nc.vector.scalar_tensor_tensor(
        sq[:], xc[:], 1.0, xc[:],
        mybir.AluOpType.mult, mybir.AluOpType.mult, accum_out=varsum[:],
    )
    vpe = work.tile([64, 1], F32)
    nc.vector.tensor_scalar(
        vpe[:], varsum[:], 1.0 / H, 1.0e-5,
        mybir.AluOpType.mult, mybir.AluOpType.add,
    )
    std = work.tile([64, 1], F32)
    nc.scalar.activation(std[:], vpe[:], AF.Sqrt)
    r0 = work.tile([64, 1], F32)
    nc.vector.reciprocal(r0[:], std[:])
    # one Newton polish of rsqrt: r = r0 * (1.5 - 0.5 * vpe * r0^2)
    ta = work.tile([64, 1], F32)
    nc.vector.tensor_mul(ta[:], r0[:], r0[:])
    tb = work.tile([64, 1], F32)
    nc.vector.tensor_mul(tb[:], ta[:], vpe[:])
    tc_ = work.tile([64, 1], F32)
    nc.vector.tensor_scalar(
        tc_[:], tb[:], -0.5, 1.5, mybir.AluOpType.mult, mybir.AluOpType.add
    )
    rinv = work.tile([64, 1], F32)
    nc.vector.tensor_mul(rinv[:], r0[:], tc_[:])
    t1 = work.tile([64, H], F32)
    nc.vector.scalar_tensor_tensor(
        t1[:], xc[:], rinv[:], gbc_sb[:],
        mybir.AluOpType.mult, mybir.AluOpType.mult,
    )
    out_sb = work.tile([64, H], F32)
    nc.vector.tensor_add(out_sb[:], t1[:], bbc_sb[:])
    nc.sync.dma_start(d["out"], out_sb[:])


def build_nc():
    nc = bacc.Bacc(
        "TRN2", target_bir_lowering=False, debug=False, num_devices=NCORES
    )
    d = {}
    d["textT"] = nc.dram_tensor("textT", [H, L], BF, kind="ExternalInput").ap()
    d["textn"] = nc.dram_tensor("textn", [L, H], BF, kind="ExternalInput").ap()
    d["meta"] = nc.dram_tensor("meta", [M, H], F32, kind="ExternalInput").ap()
    d["metaT"] = nc.dram_tensor("metaT", [H, M], BF, kind="ExternalInput").ap()
    d["mask"] = nc.dram_tensor("mask", [1, L], I32, kind="ExternalInput").ap()
    d["wqt2"] = nc.dram_tensor("wqt2", [H, 128], BF, kind="ExternalInput").ap()
    d["wk2"] = nc.dram_tensor("wk2", [H, 128], BF, kind="ExternalInput").ap()
    d["bias2"] = nc.dram_tensor("bias2", [128, 1], F32, kind="ExternalInput").ap()
    d["lt"] = nc.dram_tensor("lt", [128, NP * 64], BF, kind="ExternalInput").ap()
    d["vvt"] = nc.dram_tensor("vvt", [H, H], BF, kind="ExternalInput").ap()
    d["vvb"] = nc.dram_tensor("vvb", [1, H], BF, kind="ExternalInput").ap()
    d["ones64"] = nc.dram_tensor("ones64", [1, 64], BF, kind="ExternalInput").ap()
    d["id64"] = nc.dram_tensor("id64", [64, 64], BF, kind="ExternalInput").ap()
    d["gbc"] = nc.dram_tensor("gbc", [M, H], F32, kind="ExternalInput").ap()
    d["bbc"] = nc.dram_tensor("bbc", [M, H], F32, kind="ExternalInput").ap()
    d["out"] = nc.dram_tensor("out", [M, H], F32, kind="ExternalOutput").ap()

    with tile.TileContext(nc) as tc, ExitStack() as ctx:
        _emit(ctx, tc, nc, d)
    nc.compile()
    return nc


def make_in_maps(inputs):
    """Host-side shard + weight-layout prep. Returns list of 8 per-core maps."""
    meta = np.asarray(inputs["meta_tokens"], np.float32)
    text = np.asarray(inputs["text_tokens"], np.float32)
    mask = np.asarray(inputs["attention_mask"], np.int32)
    wq_w = np.asarray(inputs["Wq_w"], np.float32)
    wq_b = np.asarray(inputs["Wq_b"], np.float32)
    wk_w = np.asarray(inputs["Wk_w"], np.float32)
    wk_b = np.asarray(inputs["Wk_b"], np.float32)
    v_w = np.asarray(inputs["v_w"], np.float32)
    vv_w = np.asarray(inputs["Vv_w"], np.float32)
    vv_b = np.asarray(inputs["Vv_b"], np.float32)
    ln_g = np.asarray(inputs["ln_g"], np.float32)
    ln_b = np.asarray(inputs["ln_b"], np.float32)

    wqt2 = np.ascontiguousarray(
        np.concatenate([wq_w.T, wq_w.T], axis=1)
    ).astype(BF16)
    wk2 = np.ascontiguousarray(
        np.concatenate([wk_w.T, wk_w.T], axis=1)
    ).astype(BF16)
    bias2 = np.tile(wq_b + wk_b, 2)[:, None].astype(np.float32)
    lt = np.zeros((128, NP * 64), np.float32)
    for p in range(NP):
        lt[0:64, p * 64 + 2 * p] = v_w[0]
        lt[64:128, p * 64 + 2 * p + 1] = v_w[0]
    lt = lt.astype(BF16)
    vvt = np.ascontiguousarray(vv_w.T).astype(BF16)
    vvb = vv_b[None, :].astype(BF16)
    ones64 = np.ones((1, 64), BF16)
    id64 = np.eye(64, dtype=np.float32).astype(BF16)
    gbc = np.ascontiguousarray(np.broadcast_to(ln_g, (M, H))).astype(np.float32)
    bbc = np.ascontiguousarray(np.broadcast_to(ln_b, (M, H))).astype(np.float32)

    shared = dict(
        wqt2=wqt2, wk2=wk2, bias2=bias2, lt=lt, vvt=vvt, vvb=vvb,
        ones64=ones64, id64=id64, gbc=gbc, bbc=bbc,
    )
    in_maps = []
    for i in range(NCORES):
        m = dict(shared)
        m["textT"] = np.ascontiguousarray(text[i].T).astype(BF16)
        m["textn"] = np.ascontiguousarray(text[i]).astype(BF16)
        m["meta"] = np.ascontiguousarray(meta[i])
        m["metaT"] = np.ascontiguousarray(meta[i].T).astype(BF16)
        m["mask"] = np.ascontiguousarray(mask[i][None, :])
        in_maps.append(m)
    return in_maps


_cache = {}


def run(inputs, trace=False):
    if "nc" not in _cache:
        _cache["nc"] = build_nc()
    nc = _cache["nc"]
    in_maps = make_in_maps(inputs)
    res = run_bass_kernel_spmd(
        nc, in_maps, core_ids=list(range(NCORES)), trace=trace
    )
    out = np.stack(
        [np.asarray(res.results[i]["out"], np.float32) for i in range(NCORES)],
        axis=0,
    )
    return out, res


def kernel(**inputs):
    out, _ = run(inputs, trace=False)
    return out


# revision 2
# speedup vs baseline: 1.1883x; 1.1883x over previous
"""AdditiveUniAttention kernel for 8 TRN2 NeuronCores.

Strategy: pure data-parallel over B (8 batch elements, 8 cores, no collectives).

Per-core math (b fixed):
  QeT2[j*64+a, m] = sum_h meta[m,h] Wq_w[a,h]          (dup over j=0,1)
  Qbias[j*64+a, p] = QeT2[j*64+a, 2p+j] + Wq_b[a] + Wk_b[a]
  Ke2[j*64+a, l]  = sum_h text[l,h] Wk_w[a,h]          (dup over j=0,1)
  T_p[(j,a), l]   = tanh(Ke2[(j,a), l] + Qbias[(j,a), p])       (ScalarE bias)
  scores[m, l]    = sum_a v[a] T_{m//2}[(m%2,a), l]    (block-column lhsT)
                    + (mask[l]-1)*1e30                  (rank-1 matmul)
  exp = exp(scores) (f32 PSUM -> bf16 SBUF), sumexp via accum_out
  D[m, h]   = sum_l exp^T[l, m] text[l, h]   (PE transposes of exp rows)
  context   = (D / sumexp) @ Vv_w^T + Vv_b   (transposed weights from host)
  out       = LayerNorm(meta + context) * g + b

Matmuls all run bf16 (1 cyc/col on PE vs 4 for f32) with f32 PSUM accumulation.
Softmax/LN math in f32.  Weight-layout prep (transposes, bf16 casts, the
block-column v matrix) happens host-side in numpy; all activation-tensor
arithmetic happens on device.
"""

import numpy as np
import ml_dtypes
from contextlib import ExitStack

import concourse.bass as bass
import concourse.tile as tile
import concourse.mybir as mybir
from concourse import bacc
from concourse.bass_utils import run_bass_kernel_spmd

BF16 = ml_dtypes.bfloat16
F32 = mybir.dt.float32
BF = mybir.dt.bfloat16
I32 = mybir.dt.int32

B, M, L, H, A = 8, 64, 2048, 768, 64
NCORES = 8
LH = L // 2          # l-half size (1024)
NP = M // 2          # m-pairs (32)
HC = H // 128        # h chunks (6)
AF = mybir.ActivationFunctionType


def _emit(ctx, tc, nc, d):
    """Emit the per-core kernel body under TileContext tc."""
    const = ctx.enter_context(tc.tile_pool(name="const", bufs=1))
    work = ctx.enter_context(tc.tile_pool(name="work", bufs=1))
    tpool = ctx.enter_context(tc.tile_pool(name="tpool", bufs=3))
    pbig = ctx.enter_context(tc.tile_pool(name="pbig", bufs=2, space="PSUM"))
    psml = ctx.enter_context(tc.tile_pool(name="psml", bufs=2, space="PSUM"))
    pout = ctx.enter_context(tc.tile_pool(name="pout", bufs=1, space="PSUM"))

    # --- dummy tanh: pull the exp_and_others ACT table load to t=0
    dmy = const.tile([128, 1], F32)
    nc.vector.memset(dmy[:], 0.0)
    nc.scalar.activation(dmy[:], dmy[:], AF.Tanh)

    # --- critical-path DMAs first: textT half 0
    textT_sb = []
    for hc in range(HC):
        t_tT = const.tile([128, L], BF, name=f"textT{hc}", tag=f"textT{hc}")
        textT_sb.append(t_tT)
    for hc in range(HC):
        nc.sync.dma_start(
            textT_sb[hc][:, 0:LH], d["textT"][hc * 128:(hc + 1) * 128, 0:LH]
        )

    # --- small weights / meta
    wk2_sb = const.tile([128, HC * 128], BF)
    nc.sync.dma_start(
        wk2_sb[:].rearrange("p (c m) -> p c m", c=HC),
        d["wk2"].rearrange("(c p) m -> p c m", p=128),
    )
    wqt2_sb = const.tile([128, HC * 128], BF)
    nc.sync.dma_start(
        wqt2_sb[:].rearrange("p (c m) -> p c m", c=HC),
        d["wqt2"].rearrange("(c p) m -> p c m", p=128),
    )
    metaT_sb = const.tile([128, HC * 64], BF)
    nc.sync.dma_start(
        metaT_sb[:].rearrange("p (c m) -> p c m", c=HC),
        d["metaT"].rearrange("(c p) m -> p c m", p=128),
    )
    lt_sb = const.tile([128, NP * 64], BF)
    nc.sync.dma_start(lt_sb[:], d["lt"][:, :])
    bias2_sb = const.tile([128, 1], F32)
    nc.sync.dma_start(bias2_sb[:], d["bias2"][:, :])
    ones64_sb = const.tile([1, 64], BF)
    nc.sync.dma_start(ones64_sb[:], d["ones64"][:, :])
    id64_sb = const.tile([64, 64], BF)
    nc.sync.dma_start(id64_sb[:], d["id64"][:, :])
    mask_sb = work.tile([1, L], I32)
    nc.sync.dma_start(mask_sb[:], d["mask"][:, :])

    # --- textT half 1, then the natural-layout text, then tail weights
    for hc in range(HC):
        nc.sync.dma_start(
            textT_sb[hc][:, LH:L], d["textT"][hc * 128:(hc + 1) * 128, LH:L]
        )
    tb_sb = const.tile([128, 16 * H], BF)   # text natural: [l%128, (l//128, h)]
    for g in range(4):
        nc.sync.dma_start(
            tb_sb[:, g * 4 * H:(g + 1) * 4 * H].rearrange("p (c m) -> p c m", c=4),
            d["textn"][g * 512:(g + 1) * 512, :].rearrange("(c p) m -> p c m", p=128),
        )
    vvt_sb = const.tile([128, HC * H], BF)
    nc.sync.dma_start(
        vvt_sb[:].rearrange("p (c m) -> p c m", c=HC),
        d["vvt"].rearrange("(c p) m -> p c m", p=128),
    )
    vvb_sb = const.tile([1, H], BF)
    nc.sync.dma_start(vvb_sb[:], d["vvb"][:, :])
    meta_sb = const.tile([64, H], F32)
    nc.sync.dma_start(meta_sb[:], d["meta"][:, :])
    gbc_sb = const.tile([64, H], F32)
    nc.sync.dma_start(gbc_sb[:], d["gbc"][:, :])
    bbc_sb = const.tile([64, H], F32)
    nc.sync.dma_start(bbc_sb[:], d["bbc"][:, :])

    # --- Qe path: QeT2 = wqt2^T-chunks @ metaT-chunks, then Qbias assembly
    qe_ps = psml.tile([128, 64], F32, tag="sm")
    for hc in range(HC):
        nc.tensor.matmul(
            qe_ps[:],
            lhsT=wqt2_sb[:, hc * 128:(hc + 1) * 128],
            rhs=metaT_sb[:, hc * 64:(hc + 1) * 64],
            start=(hc == 0),
            stop=(hc == HC - 1),
        )
    qet2_sb = work.tile([128, 64], F32)
    nc.vector.tensor_copy(qet2_sb[:], qe_ps[:])
    qbias = work.tile([128, NP], F32)
    nc.vector.tensor_scalar(
        qbias[0:64, :], qet2_sb[0:64, 0:64:2], bias2_sb[0:64, :], None,
        mybir.AluOpType.add,
    )
    nc.vector.tensor_scalar(
        qbias[64:128, :], qet2_sb[64:128, 1:64:2], bias2_sb[64:128, :], None,
        mybir.AluOpType.add,
    )

    # --- main loop over l-halves
    exp_bf = work.tile([64, L], BF)
    ke2_sb = work.tile([128, L], F32)
    attnT_sb = work.tile([128, 16 * 64], BF)
    d_ps = pout.tile([64, H], F32, tag="o")
    sumexp = []
    for half in range(2):
        l0 = half * LH
        # Ke2 for this half: accumulate over h-chunks into PSUM, copy to SBUF
        ke_ps = pbig.tile([128, LH], F32, tag="big", name=f"ke_ps{half}")
        for hc in range(HC):
            for j in range(2):
                nc.tensor.matmul(
                    ke_ps[:, j * 512:(j + 1) * 512],
                    lhsT=wk2_sb[:, hc * 128:(hc + 1) * 128],
                    rhs=textT_sb[hc][:, l0 + j * 512:l0 + (j + 1) * 512],
                    start=(hc == 0),
                    stop=(hc == HC - 1),
                )
        nc.vector.tensor_copy(ke2_sb[:, l0:l0 + LH], ke_ps[:])

        # tanh + score matmuls
        sc_ps = pbig.tile([64, LH], F32, tag="big", name=f"sc_ps{half}")
        for p in range(NP):
            tt = tpool.tile([128, LH], BF, name="tt", tag="tt")
            nc.scalar.activation(
                tt[:], ke2_sb[:, l0:l0 + LH], AF.Tanh, bias=qbias[:, p:p + 1]
            )
            for j in range(2):
                nc.tensor.matmul(
                    sc_ps[:, j * 512:(j + 1) * 512],
                    lhsT=lt_sb[:, p * 64:(p + 1) * 64],
                    rhs=tt[:, j * 512:(j + 1) * 512],
                    start=(p == 0),
                    stop=False,
                )
        if half == 0:
            # mask row prep (off ACT critical path; emitted late so the DVE
            # runs the Ke2/Qbias copies first)
            mrow_f = work.tile([1, L], F32)
            nc.vector.tensor_copy(mrow_f[:], mask_sb[:])
            mrow_s = work.tile([1, L], F32)
            nc.vector.tensor_scalar(
                mrow_s[:], mrow_f[:], -1.0, 1.0e30,
                mybir.AluOpType.add, mybir.AluOpType.mult,
            )
            mrow_bf = work.tile([1, L], BF)
            nc.vector.tensor_copy(mrow_bf[:], mrow_s[:])
        for j in range(2):
            nc.tensor.matmul(
                sc_ps[:, j * 512:(j + 1) * 512],
                lhsT=ones64_sb[:],
                rhs=mrow_bf[0:1, l0 + j * 512:l0 + (j + 1) * 512],
                start=False,
                stop=True,
            )

        # exp (+ row sums), straight out of PSUM
        se = work.tile([64, 1], F32, name=f"se{half}", tag=f"se{half}")
        nc.scalar.activation(
            exp_bf[:, l0:l0 + LH], sc_ps[:], AF.Exp, accum_out=se[:]
        )
        sumexp.append(se)

        # attn^T chunks (PE transpose) + context matmul D += attnT^T @ text
        for k in range(half * 8, half * 8 + 8):
            tp = psml.tile([128, 64], BF, tag="sm", name="tp")
            nc.tensor.transpose(tp[:], exp_bf[:, k * 128:(k + 1) * 128], id64_sb[:])
            nc.vector.tensor_copy(attnT_sb[:, k * 64:(k + 1) * 64], tp[:])
            for n0, nw in ((0, 512), (512, 256)):
                nc.tensor.matmul(
                    d_ps[:, n0:n0 + nw],
                    lhsT=attnT_sb[:, k * 64:(k + 1) * 64],
                    rhs=tb_sb[:, k * H + n0:k * H + n0 + nw],
                    start=(k == 0),
                    stop=(k == 15),
                )

    # --- epilogue
    s_all = work.tile([64, 1], F32)
    nc.vector.tensor_add(s_all[:], sumexp[0][:], sumexp[1][:])
    sinv = work.tile([64, 1], F32)
    nc.vector.reciprocal(sinv[:], s_all[:])
    d_sb = work.tile([64, H], BF)
    nc.vector.tensor_scalar(
        d_sb[:], d_ps[:], sinv[:], None, mybir.AluOpType.mult
    )
    dt_sb = work.tile([128, HC * 64], BF)
    for hc in range(HC):
        dtp = psml.tile([128, 64], BF, tag="sm", name="dtp")
        nc.tensor.transpose(dtp[:], d_sb[:, hc * 128:(hc + 1) * 128], id64_sb[:])
        nc.vector.tensor_copy(dt_sb[:, hc * 64:(hc + 1) * 64], dtp[:])
    ctx_ps = pout.tile([64, H], F32, tag="o")
    for hc in range(HC):
        for n0, nw in ((0, 512), (512, 256)):
            nc.tensor.matmul(
                ctx_ps[:, n0:n0 + nw],
                lhsT=dt_sb[:, hc * 64:(hc + 1) * 64],
                rhs=vvt_sb[:, hc * H + n0:hc * H + n0 + nw],
                start=(hc == 0),
                stop=False,
            )
    for n0, nw in ((0, 512), (512, 256)):
        nc.tensor.matmul(
            ctx_ps[:, n0:n0 + nw],
            lhsT=ones64_sb[:],
            rhs=vvb_sb[0:1, n0:n0 + nw],
            start=False,
            stop=True,
        )

    # x = meta + context;  LayerNorm
    x_sb = work.tile([64, H], F32)
    nc.vector.tensor_add(x_sb[:], meta_sb[:], ctx_ps[:])
    xsum = work.tile([64, 1], F32)
    nc.vector.reduce_sum(xsum[:], x_sb[:], axis=mybir.AxisListType.X)
    mu = work.tile([64, 1], F32)
    nc.vector.tensor_scalar(mu[:], xsum[:], 1.0 / H, None, mybir.AluOpType.mult)
    xc = work.tile([64, H], F32)
    nc.vector.tensor_scalar(xc[:], x_sb[:], mu[:], None, mybir.AluOpType.subtract)
    sq = work.tile([64, H], F32)
    varsum = work.tile([64, 1], F32)
    nc.vector.scalar_tensor_tensor(
        sq[:], xc[:], 1.0, xc[:],
        mybir.AluOpType.mult, mybir.AluOpType.mult, accum_out=varsum[:],
    )
    vpe = work.tile([64, 1], F32)
    nc.vector.tensor_scalar(
        vpe[:], varsum[:], 1.0 / H, 1.0e-5,
        mybir.AluOpType.mult, mybir.AluOpType.add,
    )
    std = work.tile([64, 1], F32)
    nc.scalar.activation(std[:], vpe[:], AF.Sqrt)
    r0 = work.tile([64, 1], F32)
    nc.vector.reciprocal(r0[:], std[:])
    # one Newton polish of rsqrt: r = r0 * (1.5 - 0.5 * vpe * r0^2)
    ta = work.tile([64, 1], F32)
    nc.vector.tensor_mul(ta[:], r0[:], r0[:])
    tb = work.tile([64, 1], F32)
    nc.vector.tensor_mul(tb[:], ta[:], vpe[:])
    tc_ = work.tile([64, 1], F32)
    nc.vector.tensor_scalar(
        tc_[:], tb[:], -0.5, 1.5, mybir.AluOpType.mult, mybir.AluOpType.add
    )
    rinv = work.tile([64, 1], F32)
    nc.vector.tensor_mul(rinv[:], r0[:], tc_[:])
    t1 = work.tile([64, H], F32)
    nc.vector.scalar_tensor_tensor(
        t1[:], xc[:], rinv[:], gbc_sb[:],
        mybir.AluOpType.mult, mybir.AluOpType.mult,
    )
    out_sb = work.tile([64, H], F32)
    nc.vector.tensor_add(out_sb[:], t1[:], bbc_sb[:])
    nc.sync.dma_start(d["out"], out_sb[:])


def build_nc():
    nc = bacc.Bacc(
        "TRN2", target_bir_lowering=False, debug=False, num_devices=NCORES
    )
    d = {}
    d["textT"] = nc.dram_tensor("textT", [H, L], BF, kind="ExternalInput").ap()
    d["textn"] = nc.dram_tensor("textn", [L, H], BF, kind="ExternalInput").ap()
    d["meta"] = nc.dram_tensor("meta", [M, H], F32, kind="ExternalInput").ap()
    d["metaT"] = nc.dram_tensor("metaT", [H, M], BF, kind="ExternalInput").ap()
    d["mask"] = nc.dram_tensor("mask", [1, L], I32, kind="ExternalInput").ap()
    d["wqt2"] = nc.dram_tensor("wqt2", [H, 128], BF, kind="ExternalInput").ap()
    d["wk2"] = nc.dram_tensor("wk2", [H, 128], BF, kind="ExternalInput").ap()
    d["bias2"] = nc.dram_tensor("bias2", [128, 1], F32, kind="ExternalInput").ap()
    d["lt"] = nc.dram_tensor("lt", [128, NP * 64], BF, kind="ExternalInput").ap()
    d["vvt"] = nc.dram_tensor("vvt", [H, H], BF, kind="ExternalInput").ap()
    d["vvb"] = nc.dram_tensor("vvb", [1, H], BF, kind="ExternalInput").ap()
    d["ones64"] = nc.dram_tensor("ones64", [1, 64], BF, kind="ExternalInput").ap()
    d["id64"] = nc.dram_tensor("id64", [64, 64], BF, kind="ExternalInput").ap()
    d["gbc"] = nc.dram_tensor("gbc", [M, H], F32, kind="ExternalInput").ap()
    d["bbc"] = nc.dram_tensor("bbc", [M, H], F32, kind="ExternalInput").ap()
    d["out"] = nc.dram_tensor("out", [M, H], F32, kind="ExternalOutput").ap()

    with tile.TileContext(nc) as tc, ExitStack() as ctx:
        _emit(ctx, tc, nc, d)
    nc.compile()
    return nc


def make_in_maps(inputs):
    """Host-side shard + weight-layout prep. Returns list of 8 per-core maps."""
    meta = np.asarray(inputs["meta_tokens"], np.float32)
    text = np.asarray(inputs["text_tokens"], np.float32)
    mask = np.asarray(inputs["attention_mask"], np.int32)
    wq_w = np.asarray(inputs["Wq_w"], np.float32)
    wq_b = np.asarray(inputs["Wq_b"], np.float32)
    wk_w = np.asarray(inputs["Wk_w"], np.float32)
    wk_b = np.asarray(inputs["Wk_b"], np.float32)
    v_w = np.asarray(inputs["v_w"], np.float32)
    vv_w = np.asarray(inputs["Vv_w"], np.float32)
    vv_b = np.asarray(inputs["Vv_b"], np.float32)
    ln_g = np.asarray(inputs["ln_g"], np.float32)
    ln_b = np.asarray(inputs["ln_b"], np.float32)

    wqt2 = np.ascontiguousarray(
        np.concatenate([wq_w.T, wq_w.T], axis=1)
    ).astype(BF16)
    wk2 = np.ascontiguousarray(
        np.concatenate([wk_w.T, wk_w.T], axis=1)
    ).astype(BF16)
    bias2 = np.tile(wq_b + wk_b, 2)[:, None].astype(np.float32)
    lt = np.zeros((128, NP * 64), np.float32)
    for p in range(NP):
        lt[0:64, p * 64 + 2 * p] = v_w[0]
        lt[64:128, p * 64 + 2 * p + 1] = v_w[0]
    lt = lt.astype(BF16)
    vvt = np.ascontiguousarray(vv_w.T).astype(BF16)
    vvb = vv_b[None, :].astype(BF16)
    ones64 = np.ones((1, 64), BF16)
    id64 = np.eye(64, dtype=np.float32).astype(BF16)
    gbc = np.ascontiguousarray(np.broadcast_to(ln_g, (M, H))).astype(np.float32)
    bbc = np.ascontiguousarray(np.broadcast_to(ln_b, (M, H))).astype(np.float32)

    shared = dict(
        wqt2=wqt2, wk2=wk2, bias2=bias2, lt=lt, vvt=vvt, vvb=vvb,
        ones64=ones64, id64=id64, gbc=gbc, bbc=bbc,
    )
    in_maps = []
    for i in range(NCORES):
        m = dict(shared)
        m["textT"] = np.ascontiguousarray(text[i].T).astype(BF16)
        m["textn"] = np.ascontiguousarray(text[i]).astype(BF16)
        m["meta"] = np.ascontiguousarray(meta[i])
        m["metaT"] = np.ascontiguousarray(meta[i].T).astype(BF16)
        m["mask"] = np.ascontiguousarray(mask[i][None, :])
        in_maps.append(m)
    return in_maps


_cache = {}


def run(inputs, trace=False, tmpdir=None):
    if "nc" not in _cache:
        _cache["nc"] = build_nc()
    nc = _cache["nc"]
    in_maps = make_in_maps(inputs)
    res = run_bass_kernel_spmd(
        nc, in_maps, core_ids=list(range(NCORES)), trace=trace, tmpdir=tmpdir
    )
    out = np.stack(
        [np.asarray(res.results[i]["out"], np.float32) for i in range(NCORES)],
        axis=0,
    )
    return out, res


def kernel(**inputs):
    out, _ = run(inputs, trace=False)
    return out
